# revision 5
# baseline (speedup 1.0000x reference)
"""Trainium2 Bass kernel for nn_CausalAttentionLayer (sparse_attention).

Reference computes, per batch b (B=32, Nq=Nk=1024, C=128, CM=256):
    S = Q @ K^T                      # [1024, 1024], no 1/sqrt(d) scale
    P = softmax(S, axis=-1) * strict_lower_mask   # mask AFTER full-row softmax
    O = P @ V                        # [1024, 256]

Sharding: data-parallel over batch, 4 batches per core on 8 NeuronCores.

Per-core device algorithm (per batch), in the transposed S^T = K Q^T layout
(k on partitions, q on free axis) so no on-device transposes are needed:
  - Q^T, K^T are prepared on host as [C, Nq] and split into bf16 hi/lo pairs;
    S^T block i = Kh_i^T Qh + Kl_i^T Qh + Kh_i^T Ql  (error ~2^-18, fp32 PSUM)
  - P^T_i = exp(S^T_i) on ScalarE, written as float32r (rounded) to SBUF
  - l[q] = sum_k exp(S[q,k]) over ALL k via ones-matmuls into PSUM [2, 512]
  - diag blocks multiplied by strict-upper mask (q_local > k_local) on DVE
  - O_j = sum_{i<=j} P^T_{i,jblock}^T V_i  (causal: only 36 of 64 block mms)
  - O_j *= 1/l (per-partition scalar), DMA out.
"""

import os
import sys
from contextlib import ExitStack

import numpy as np

sys.path.insert(0, "/opt/trn_rl_repo")

import ml_dtypes  # noqa: E402

import concourse.tile as tile  # noqa: E402
from concourse import bacc, mybir  # noqa: E402
from concourse.bass_utils import run_bass_kernel_spmd  # noqa: E402

N_CORES = 8
B_TOTAL = 32
NQ = 1024
C = 128
CM = 256
NBLK = NQ // 128  # 8

_cache = {}


def build(b_core):
    """Build + compile the per-core Bass program processing b_core batches."""
    f32 = mybir.dt.float32
    f32r = mybir.dt.float32r
    bf16 = mybir.dt.bfloat16

    nc = bacc.Bacc(
        "TRN2", target_bir_lowering=False, debug=False, num_devices=N_CORES
    )
    qh = nc.dram_tensor("qh", [b_core, C, NQ], bf16, kind="ExternalInput").ap()
    ql = nc.dram_tensor("ql", [b_core, C, NQ], bf16, kind="ExternalInput").ap()
    kh = nc.dram_tensor("kh", [b_core, C, NQ], bf16, kind="ExternalInput").ap()
    kl = nc.dram_tensor("kl", [b_core, C, NQ], bf16, kind="ExternalInput").ap()
    v = nc.dram_tensor("v", [b_core, NQ, CM], f32r, kind="ExternalInput").ap()
    mask = nc.dram_tensor("mask", [128, 128], f32r, kind="ExternalInput").ap()
    ones = nc.dram_tensor("ones", [128, 1], f32r, kind="ExternalInput").ap()
    out = nc.dram_tensor("out", [b_core, NQ, CM], f32, kind="ExternalOutput").ap()

    with tile.TileContext(nc) as tc, ExitStack() as ctx:
        const_pool = ctx.enter_context(tc.tile_pool(name="const", bufs=1))
        qk_pool = ctx.enter_context(tc.tile_pool(name="qk", bufs=8))
        v_pool = ctx.enter_context(tc.tile_pool(name="vp", bufs=16))
        pt_pool = ctx.enter_context(tc.tile_pool(name="pt", bufs=16))
        ptm_pool = ctx.enter_context(tc.tile_pool(name="ptm", bufs=2))
        l_pool = ctx.enter_context(tc.tile_pool(name="lsb", bufs=2))
        r_pool = ctx.enter_context(tc.tile_pool(name="rsb", bufs=2))
        o_pool = ctx.enter_context(tc.tile_pool(name="osb", bufs=4))
        ps_s = ctx.enter_context(tc.tile_pool(name="ps_s", bufs=2, space="PSUM"))
        ps_o = ctx.enter_context(tc.tile_pool(name="ps_o", bufs=1, space="PSUM"))
        ps_l = ctx.enter_context(tc.tile_pool(name="ps_l", bufs=1, space="PSUM"))
        ps_r = ctx.enter_context(tc.tile_pool(name="ps_r", bufs=1, space="PSUM"))

        mask_sb = const_pool.tile([128, 128], f32r)
        nc.sync.dma_start(mask_sb[:], mask)
        ones_sb = const_pool.tile([128, 1], f32r)
        nc.sync.dma_start(ones_sb[:], ones)
        # plain-f32 scalar 1.0 for the l-redistribution transpose matmuls
        onesf_sb = const_pool.tile([1, 1], f32)
        nc.vector.memset(onesf_sb[:], 1.0)

        for b in range(b_core):
            qh_sb = qk_pool.tile([C, NQ], bf16, tag="qh")
            nc.sync.dma_start(qh_sb[:], qh[b, :, :])
            ql_sb = qk_pool.tile([C, NQ], bf16, tag="ql")
            nc.sync.dma_start(ql_sb[:], ql[b, :, :])
            kh_sb = qk_pool.tile([C, NQ], bf16, tag="kh")
            nc.sync.dma_start(kh_sb[:], kh[b, :, :])
            kl_sb = qk_pool.tile([C, NQ], bf16, tag="kl")
            nc.sync.dma_start(kl_sb[:], kl[b, :, :])
            v_sb = []
            for i in range(NBLK):
                vt = v_pool.tile([128, CM], f32r, tag="v")
                nc.sync.dma_start(vt[:], v[b, 128 * i : 128 * (i + 1), :])
                v_sb.append(vt)

            ptm = ptm_pool.tile([128, NQ], f32r, tag="ptm")
            psl = ps_l.tile([1, NQ], f32)  # spans 2 PSUM banks; mms write halves
            pt = []
            for i in range(NBLK):
                kslc = slice(128 * i, 128 * (i + 1))
                s_ps = ps_s.tile([128, NQ], f32, tag="s")
                for h in (0, 1):
                    qslc = slice(512 * h, 512 * (h + 1))
                    terms = (
                        (kh_sb, qh_sb),
                        (kl_sb, qh_sb),
                        (kh_sb, ql_sb),
                    )
                    for t, (kt_, qt_) in enumerate(terms):
                        nc.tensor.matmul(
                            s_ps[:, qslc],
                            kt_[:, kslc],
                            qt_[:, qslc],
                            start=(t == 0),
                            stop=(t == 2),
                        )
                pt_i = pt_pool.tile([128, NQ], f32r, tag="pt")
                nc.scalar.activation(
                    pt_i[:], s_ps[:], mybir.ActivationFunctionType.Exp
                )
                pt.append(pt_i)
                # accumulate full-row sums l into [2, 512] (q = 512*h + q')
                for h in (0, 1):
                    nc.tensor.matmul(
                        psl[:, 512 * h : 512 * (h + 1)],
                        ones_sb[:],
                        pt_i[:, 512 * h : 512 * (h + 1)],
                        start=(i == 0),
                        stop=(i == NBLK - 1),
                        skip_group_check=True,
                    )
                # strict-upper mask for the diagonal block (keep q_local > k_local)
                nc.vector.tensor_mul(ptm[:, kslc], pt_i[:, kslc], mask_sb[:])

            # redistribute l [2, 512] -> r [128, 8] = 1/l per q-partition
            l_sb = l_pool.tile([1, NQ], f32, tag="l")
            nc.vector.tensor_copy(l_sb[:], psl[:])
            psr = ps_r.tile([128, 8], f32)
            for j in range(NBLK):
                nc.tensor.matmul(
                    psr[:, j : j + 1],
                    l_sb[:, 128 * j : 128 * (j + 1)],
                    onesf_sb[0:1, :],
                    start=True,
                    stop=True,
                )
            r_sb = r_pool.tile([128, 8], f32, tag="r")
            nc.vector.reciprocal(r_sb[:], psr[:])

            # causal PV: O_j = sum_{i<=j} PT_{i}[:, jblock].T @ V_i, then * r
            for j in range(NBLK):
                jslc = slice(128 * j, 128 * (j + 1))
                o_ps = ps_o.tile([128, CM], f32, tag="o")
                for i in range(j + 1):
                    lhsT = ptm[:, jslc] if i == j else pt[i][:, jslc]
                    nc.tensor.matmul(
                        o_ps[:],
                        lhsT,
                        v_sb[i][:],
                        start=(i == 0),
                        stop=(i == j),
                    )
                o_sb = o_pool.tile([128, CM], f32, tag="o_sb")
                nc.vector.tensor_scalar_mul(o_sb[:], o_ps[:], r_sb[:, j : j + 1])
                nc.sync.dma_start(out[b, jslc, :], o_sb[:])

    nc.compile()
    return nc


def host_prep(query, key, value):
    """Full inputs -> per-core in_maps (host-side layout prep + sharding)."""
    q = np.ascontiguousarray(np.asarray(query, dtype=np.float32)).reshape(
        B_TOTAL, NQ, C
    )
    k = np.ascontiguousarray(np.asarray(key, dtype=np.float32)).reshape(
        B_TOTAL, NQ, C
    )
    v = np.ascontiguousarray(np.asarray(value, dtype=np.float32)).reshape(
        B_TOTAL, NQ, CM
    )
    qt = np.ascontiguousarray(q.transpose(0, 2, 1))  # [B, C, NQ]
    kt = np.ascontiguousarray(k.transpose(0, 2, 1))
    bf16 = ml_dtypes.bfloat16
    qth = qt.astype(bf16)
    qtl = (qt - qth.astype(np.float32)).astype(bf16)
    kth = kt.astype(bf16)
    ktl = (kt - kth.astype(np.float32)).astype(bf16)
    mask_np = np.triu(np.ones((128, 128), dtype=np.float32), k=1)
    ones_np = np.ones((128, 1), dtype=np.float32)

    b_core = B_TOTAL // N_CORES
    in_maps = []
    for cidx in range(N_CORES):
        sl = slice(b_core * cidx, b_core * (cidx + 1))
        in_maps.append(
            {
                "qh": np.ascontiguousarray(qth[sl]),
                "ql": np.ascontiguousarray(qtl[sl]),
                "kh": np.ascontiguousarray(kth[sl]),
                "kl": np.ascontiguousarray(ktl[sl]),
                "v": np.ascontiguousarray(v[sl]),
                "mask": mask_np,
                "ones": ones_np,
            }
        )
    return in_maps


def kernel(query, key, value):
    b_core = B_TOTAL // N_CORES
    if "nc" not in _cache:
        _cache["nc"] = build(b_core)
    nc = _cache["nc"]
    in_maps = host_prep(query, key, value)
    res = run_bass_kernel_spmd(
        nc, in_maps, core_ids=list(range(N_CORES)), trace=False
    )
    out = np.concatenate([r["out"] for r in res.results], axis=0)
    return out.reshape(B_TOTAL, 32, 32, CM).astype(np.float32)


if __name__ == "__main__":
    rng = np.random.default_rng(0)
    q = rng.standard_normal((B_TOTAL, 32, 32, C), dtype=np.float32)
    k = rng.standard_normal((B_TOTAL, 32, 32, C), dtype=np.float32)
    v = rng.standard_normal((B_TOTAL, 32, 32, CM), dtype=np.float32)
    o = kernel(query=q, key=k, value=v)
    print(o.shape, o.dtype)
